# revision 21
# baseline (speedup 1.0000x reference)
"""Trainium2 Bass kernel for quantized Llama MLP (int8 gate_up -> silu*up ->
dynamic per-token requant -> int8 down_proj), tensor-parallel over 8 cores.

Sharding: column-parallel gate_up (2I split, gate/up halves aligned per shard),
row-parallel down (I split), AllReduce(max) for the dynamic per-token scale,
chunked ReduceScatter(add) on bf16 partial outputs.
"""
import sys, types
import numpy as np

if '/opt/trn_rl_repo' not in sys.path:
    sys.path.insert(0, '/opt/trn_rl_repo')

# antenv.axon_hooks is absent in this image; concourse imports it for NTFF
# profiling under axon. Register the ctypes-based hook before concourse loads.
def _ensure_ntff_hook():
    try:
        import antenv
        if "antenv.axon_hooks" not in sys.modules:
            hooks = types.ModuleType("antenv.axon_hooks")
            _h = [None]
            hooks.set_axon_ntff_profile_hook = lambda h: _h.__setitem__(0, h)
            hooks.get_axon_ntff_profile_hook = lambda: _h[0]
            sys.modules["antenv.axon_hooks"] = hooks
            antenv.axon_hooks = hooks
        import antenv.axon_hooks as hooks
        if hooks.get_axon_ntff_profile_hook() is None:
            try:
                from trn_agent_boot.trn_boot import _ntff_profile_via_ctypes
                hooks.set_axon_ntff_profile_hook(
                    _ntff_profile_via_ctypes('/opt/axon/libaxon_pjrt.so'))
            except Exception:
                pass
    except Exception:
        pass

_ensure_ntff_hook()

from concourse import bacc, tile, mybir
from concourse import bass_utils
from concourse import bass_isa

bass_utils.upload_artifacts = lambda tmpdir: tmpdir  # no bucket in container

F16 = mybir.dt.float16
F32 = mybir.dt.float32
BF16 = mybir.dt.bfloat16
MAGIC = 12582912.0  # 1.5 * 2**23: fp32 round-to-nearest-even at integers
# `a` is stored fp16 scaled by C_SCALE (folded into the up-proj scales) so
# silu(gate)*up products stay within fp16 range; round(a*127/amax) is
# invariant to the common factor, and the output dequant absorbs 1/C_SCALE.
C_SCALE = 1.0 / 64.0

NCORES = 8


class Cfg:
    def __init__(self, T, H, I, tb=512):
        self.T, self.H, self.I = T, H, I
        self.TB = tb                      # token block width for mm1 rhs
        self.NT = T // tb                 # mm1 t-blocks
        self.KH = H // 128                # mm1 contraction tiles
        nblk_tot = -(-I // 128)           # i 128-blocks, padded up
        nblk_tot = -(-nblk_tot // NCORES) * NCORES
        self.IPAD = nblk_tot * 128
        self.NPAIR = nblk_tot // NCORES   # i-blocks per core
        self.MT = T // 128                # mm2 token M-tiles
        self.HB = min(512, H)             # mm2 h block
        self.HN = H // self.HB            # mm2 h N-blocks
        self.NCH = 4                      # ReduceScatter chunks
        assert T % tb == 0 and H % 128 == 0 and self.MT % self.NCH == 0
        self.MPC = self.MT // self.NCH    # m-tiles per chunk
        self.TCH = T // self.NCH          # tokens per chunk
        self.TSL = self.TCH // NCORES     # tokens per rank slice


FULL = Cfg(2048, 4096, 11008)


def build(cfg=FULL, debug=False):
    T, H, I = cfg.T, cfg.H, cfg.I
    TB, NT, KH, NPAIR = cfg.TB, cfg.NT, cfg.KH, cfg.NPAIR
    MT, HB, HN, NCH, MPC, TSL = cfg.MT, cfg.HB, cfg.HN, cfg.NCH, cfg.MPC, cfg.TSL
    RG = [list(range(NCORES))]

    nc = bacc.Bacc("TRN2", target_bir_lowering=False, debug=debug,
                   num_devices=NCORES)
    xt_d = nc.dram_tensor("xt", [NT, 128, KH * TB], F16, kind="ExternalInput")
    wgu_d = nc.dram_tensor("wgu", [NPAIR, 2, 128, KH * 128], F16,
                           kind="ExternalInput")
    sgu_d = nc.dram_tensor("sgu", [128, NPAIR * 2], F32, kind="ExternalInput")
    wd_d = nc.dram_tensor("wd", [NPAIR, 128, H], F16, kind="ExternalInput")
    sx_d = nc.dram_tensor("sx", [1, T], F32, kind="ExternalInput")
    sdn_d = nc.dram_tensor("sdn", [1, H], F32, kind="ExternalInput")  # s_down/127
    out_d = nc.dram_tensor("out", [T // NCORES, H], F32, kind="ExternalOutput")

    with tile.TileContext(nc) as tc:
        with tc.tile_pool(name="const", bufs=1) as cpool, \
             tc.tile_pool(name="dram", bufs=1, space="DRAM") as dpool, \
             tc.tile_pool(name="psum", bufs=8, space="PSUM") as pspool, \
             tc.tile_pool(name="aq", bufs=1) as aqpool:

            # ---- constants ----
            sgu_sb = cpool.tile([128, NPAIR * 2], F32, tag="sgu")
            nc.sync.dma_start(out=sgu_sb[:, :], in_=sgu_d[:, :])
            sa_sb = cpool.tile([128, MT], F32, tag="sasb")

            aq = aqpool.tile([128, NPAIR, T], F16, tag="aq")  # a then a_q

            s_loc_d = dpool.tile([1, T], F32, tag="sloc")
            s_glob_d = dpool.tile([1, T], F32, tag="sglob")
            chunk_d = [dpool.tile([cfg.TCH, H], BF16, tag=f"chunk{c}",
                                  name=f"chunk{c}") for c in range(NCH)]
            rsout_d = [dpool.tile([TSL, H], BF16, tag=f"rsout{c}",
                                  name=f"rsout{c}") for c in range(NCH)]

            # ---- phase 1: gate_up matmul + dequant + silu*up + local amax ----
            with tc.tile_pool(name="xblk", bufs=3) as xpool, \
                 tc.tile_pool(name="wtile", bufs=2) as wpool, \
                 tc.tile_pool(name="tmp", bufs=2) as tpool, \
                 tc.tile_pool(name="rmax", bufs=1) as rpool:

                sx_row = rpool.tile([1, T], F32, tag="rows")
                nc.sync.dma_start(out=sx_row[:, :], in_=sx_d[:, :])
                sx_b = rpool.tile([128, T], F32, tag="sxb")
                nc.gpsimd.partition_broadcast(sx_b[:, :], sx_row[:, :])

                runmax = rpool.tile([128, T], F32, tag="runmax")
                nc.vector.memset(runmax[:, :], 0.0)
                runmin = rpool.tile([128, T], F32, tag="runmin")
                nc.vector.memset(runmin[:, :], 0.0)

                KHH = KH // 2
                for n in range(NT):
                    nsl = slice(n * TB, (n + 1) * TB)
                    x_h = []
                    for h2 in range(2):
                        x_t = xpool.tile([128, KHH, TB], F16, tag="xt")
                        nc.sync.dma_start(
                            out=x_t[:, :, :],
                            in_=xt_d[n][:, h2 * KHH * TB:(h2 + 1) * KHH * TB])
                        x_h.append(x_t)
                    for pr in range(NPAIR):
                        w_g = wpool.tile([128, KH, 128], F16, tag="wg")
                        nc.sync.dma_start(out=w_g[:, :, :], in_=wgu_d[pr, 0])
                        w_u = wpool.tile([128, KH, 128], F16, tag="wu")
                        nc.sync.dma_start(out=w_u[:, :, :], in_=wgu_d[pr, 1])
                        g_ps = pspool.tile([128, TB], F32, tag="ps")
                        for k in range(KH):
                            nc.tensor.matmul(g_ps[:, :], lhsT=w_g[:, k, :],
                                             rhs=x_h[k // KHH][:, k % KHH, :],
                                             start=(k == 0), stop=(k == KH - 1))
                        u_ps = pspool.tile([128, TB], F32, tag="ps")
                        for k in range(KH):
                            nc.tensor.matmul(u_ps[:, :], lhsT=w_u[:, k, :],
                                             rhs=x_h[k // KHH][:, k % KHH, :],
                                             start=(k == 0), stop=(k == KH - 1))
                        # gate = g_i32*sx[t]*sg[o]; a = gate*sigmoid(gate)*u_i32*sx*su
                        tg = tpool.tile([128, TB], F32, tag="tg")
                        nc.vector.tensor_tensor(out=tg[:, :], in0=g_ps[:, :],
                                                in1=sx_b[:, nsl],
                                                op=mybir.AluOpType.mult)
                        sig = tpool.tile([128, TB], F32, tag="sig")
                        nc.scalar.activation(sig[:, :], tg[:, :],
                                             mybir.ActivationFunctionType.Sigmoid,
                                             scale=sgu_sb[:, 2 * pr:2 * pr + 1])
                        tu = tpool.tile([128, TB], F32, tag="tu")
                        nc.vector.tensor_tensor(out=tu[:, :], in0=u_ps[:, :],
                                                in1=sx_b[:, nsl],
                                                op=mybir.AluOpType.mult)
                        t1 = tpool.tile([128, TB], F32, tag="t1")
                        nc.vector.scalar_tensor_tensor(
                            out=t1[:, :], in0=tu[:, :],
                            scalar=sgu_sb[:, 2 * pr + 1:2 * pr + 2],
                            in1=sig[:, :],
                            op0=mybir.AluOpType.mult, op1=mybir.AluOpType.mult)
                        a_sl = aq[:, pr, nsl]
                        nc.vector.scalar_tensor_tensor(
                            out=a_sl, in0=tg[:, :],
                            scalar=sgu_sb[:, 2 * pr:2 * pr + 1],
                            in1=t1[:, :],
                            op0=mybir.AluOpType.mult, op1=mybir.AluOpType.mult)
                        nc.vector.tensor_tensor(out=runmax[:, nsl],
                                                in0=runmax[:, nsl], in1=a_sl,
                                                op=mybir.AluOpType.max)
                        nc.vector.tensor_tensor(out=runmin[:, nsl],
                                                in0=runmin[:, nsl], in1=a_sl,
                                                op=mybir.AluOpType.min)

                # ---- global per-token scale ----
                pmax = rpool.tile([128, T], F32, tag="pmax")
                nc.gpsimd.partition_all_reduce(pmax[:, :], runmax[:, :], 128,
                                               bass_isa.ReduceOp.absmax)
                nc.gpsimd.partition_all_reduce(runmax[:, :], runmin[:, :], 128,
                                               bass_isa.ReduceOp.absmax)
                nc.vector.tensor_tensor(out=pmax[0:1, :], in0=pmax[0:1, :],
                                        in1=runmax[0:1, :],
                                        op=mybir.AluOpType.max)
                nc.sync.dma_start(out=s_loc_d[:, :], in_=pmax[0:1, :])
                nc.gpsimd.collective_compute(
                    "AllReduce", mybir.AluOpType.max, replica_groups=RG,
                    ins=[s_loc_d[:, :].opt()], outs=[s_glob_d[:, :].opt()])
                srow = rpool.tile([1, T], F32, tag="rows")
                nc.sync.dma_start(out=srow[:, :], in_=s_glob_d[:, :])
                nc.vector.reciprocal(srow[:, :], srow[:, :])
                nc.vector.tensor_scalar(out=srow[:, :], in0=srow[:, :],
                                        scalar1=127.0, scalar2=None,
                                        op0=mybir.AluOpType.mult)
                inv_b = rpool.tile([128, T], F32, tag="invb")
                nc.gpsimd.partition_broadcast(inv_b[:, :], srow[:, :])
                # per-m-tile s_a columns (raw gmax; /127 folded into sdn host-side)
                for m in range(MT):
                    nc.sync.dma_start(out=sa_sb[:, m:m + 1],
                                      in_=s_glob_d[0, 128 * m:128 * (m + 1)])

                # ---- quantize a -> round(a * 127/gmax), ties-to-even ----
                for pr in range(NPAIR):
                    for n in range(NT):
                        nsl = slice(n * TB, (n + 1) * TB)
                        tq = tpool.tile([128, TB], F32, tag="tq")
                        nc.vector.tensor_tensor(out=tq[:, :],
                                                in0=aq[:, pr, nsl],
                                                in1=inv_b[:, nsl],
                                                op=mybir.AluOpType.mult)
                        nc.vector.tensor_scalar(out=aq[:, pr, nsl],
                                                in0=tq[:, :],
                                                scalar1=MAGIC, scalar2=MAGIC,
                                                op0=mybir.AluOpType.add,
                                                op1=mybir.AluOpType.subtract)

            # ---- phase 2: down matmul + fused dequant + chunked RS ----
            with tc.tile_pool(name="wd", bufs=1) as wdpool, \
                 tc.tile_pool(name="ostage", bufs=1) as opool:
                sdn_row = wdpool.tile([1, H], F32, tag="sdnrow")
                nc.sync.dma_start(out=sdn_row[:, :], in_=sdn_d[:, :])
                sdn_b = wdpool.tile([128, H], F32, tag="sdnb")
                nc.gpsimd.partition_broadcast(sdn_b[:, :], sdn_row[:, :])
                wd_sb = wdpool.tile([128, NPAIR, H], F16, tag="wd")
                for blk in range(NPAIR):
                    nc.sync.dma_start(out=wd_sb[:, blk, :], in_=wd_d[blk])

                for c in range(NCH):
                    for mi in range(MPC):
                        m = c * MPC + mi
                        for n in range(HN):
                            ps = pspool.tile([128, HB], F32, tag="ps")
                            for blk in range(NPAIR):
                                nc.tensor.matmul(
                                    ps[:, :],
                                    lhsT=aq[:, blk, 128 * m:128 * (m + 1)],
                                    rhs=wd_sb[:, blk, HB * n:HB * (n + 1)],
                                    start=(blk == 0), stop=(blk == NPAIR - 1))
                            o = opool.tile([128, HB], BF16, tag="ost", bufs=8)
                            nc.vector.scalar_tensor_tensor(
                                out=o[:, :], in0=ps[:, :],
                                scalar=sa_sb[:, m:m + 1],
                                in1=sdn_b[:, HB * n:HB * (n + 1)],
                                op0=mybir.AluOpType.mult,
                                op1=mybir.AluOpType.mult)
                            nc.sync.dma_start(
                                out=chunk_d[c][128 * mi:128 * (mi + 1),
                                               HB * n:HB * (n + 1)],
                                in_=o[:, :])
                    nc.gpsimd.collective_compute(
                        "ReduceScatter", mybir.AluOpType.add, replica_groups=RG,
                        ins=[chunk_d[c][:, :].opt()],
                        outs=[rsout_d[c][:, :].opt()])
                    rb = opool.tile([TSL, H], BF16, tag="rb", bufs=1)
                    nc.sync.dma_start(out=rb[:, :], in_=rsout_d[c][:, :])
                    rf = opool.tile([TSL, H], F32, tag="rf", bufs=1)
                    nc.vector.tensor_copy(out=rf[:, :], in_=rb[:, :])
                    nc.sync.dma_start(out=out_d[c * TSL:(c + 1) * TSL, :],
                                      in_=rf[:, :])
    nc.compile()
    return nc


def prep_inputs(x_q, scale_x, w_gate_up, s_gate_up, w_down, s_down, cfg=FULL):
    """Host-side shard + relayout + exact int8->fp16 cast. Returns in_maps."""
    T, H, I = cfg.T, cfg.H, cfg.I
    TB, NT, KH, NPAIR = cfg.TB, cfg.NT, cfg.KH, cfg.NPAIR
    IPAD = cfg.IPAD

    x_q = np.asarray(x_q); scale_x = np.asarray(scale_x, np.float32)
    w_gate_up = np.asarray(w_gate_up); s_gate_up = np.asarray(s_gate_up, np.float32)
    w_down = np.asarray(w_down); s_down = np.asarray(s_down, np.float32)

    # xt: [NT, 128, KH, TB] <- xT[h, t] = x_q[t, h]
    xt = np.ascontiguousarray(
        x_q.T.astype(np.float16).reshape(KH, 128, NT, TB).transpose(2, 1, 0, 3)
    ).reshape(NT, 128, KH * TB)

    def pad_rows(w, rows):
        return np.concatenate(
            [w, np.zeros((rows - w.shape[0],) + w.shape[1:], w.dtype)], 0) \
            if w.shape[0] < rows else w

    gate = pad_rows(w_gate_up[:I], IPAD)         # [IPAD, H] int8
    up = pad_rows(w_gate_up[I:], IPAD)
    s_g = pad_rows(s_gate_up[:I], IPAD)
    s_u = pad_rows(s_gate_up[I:], IPAD)
    wdp = np.concatenate(
        [w_down, np.zeros((H, IPAD - I), w_down.dtype)], 1)  # [H, IPAD]

    gate_b = gate.reshape(IPAD // 128, 128, H)
    up_b = up.reshape(IPAD // 128, 128, H)
    wd_b = np.ascontiguousarray(wdp.T).reshape(IPAD // 128, 128, H)

    sx = scale_x.reshape(1, T)
    sdn = (s_down / (127.0 * C_SCALE)).astype(np.float32).reshape(1, H)

    in_maps = []
    for k in range(NCORES):
        bsl = slice(k * NPAIR, (k + 1) * NPAIR)
        # wgu: [NPAIR, 2, 128(h_in), KH*128(o)]; lhsT tile [h_in, o]
        wgu_k = np.empty((NPAIR, 2, 128, KH, 128), np.float16)
        for j, blkset in enumerate((gate_b[bsl], up_b[bsl])):
            # blk [128(o), H] -> [H, 128] -> [KH, 128(h_in), 128(o)] -> [h_in, KH, o]
            w = blkset.astype(np.float16).transpose(0, 2, 1)  # [NPAIR, H, 128]
            wgu_k[:, j] = w.reshape(NPAIR, KH, 128, 128).transpose(0, 2, 1, 3)
        sgu_k = np.empty((128, NPAIR * 2), np.float32)
        sgu_k[:, 0::2] = s_g[bsl.start * 128:bsl.stop * 128].reshape(NPAIR, 128).T
        sgu_k[:, 1::2] = (C_SCALE *
                          s_u[bsl.start * 128:bsl.stop * 128].reshape(NPAIR, 128).T)
        wd_k = wd_b[bsl].astype(np.float16)  # [NPAIR, 128(i_in), H]
        in_maps.append({
            "xt": xt.reshape(NT, 128, KH * TB),
            "wgu": np.ascontiguousarray(wgu_k).reshape(NPAIR, 2, 128, KH * 128),
            "sgu": sgu_k,
            "wd": np.ascontiguousarray(wd_k),
            "sx": sx, "sdn": sdn,
        })
    return in_maps


def assemble(results, cfg=FULL):
    T, H = cfg.T, cfg.H
    TSL, NCH, TCH = cfg.TSL, cfg.NCH, cfg.TCH
    full = np.empty((T, H), np.float32)
    for k in range(NCORES):
        o = results[k]["out"]
        for c in range(NCH):
            full[TCH * c + TSL * k: TCH * c + TSL * (k + 1)] = \
                o[TSL * c: TSL * (c + 1)]
    return full


_NC_CACHE = {}


def kernel(x_q, scale_x, w_gate_up, s_gate_up, w_down, s_down):
    cfg = FULL
    key = (cfg.T, cfg.H, cfg.I)
    if key not in _NC_CACHE:
        _NC_CACHE[key] = build(cfg)
    nc = _NC_CACHE[key]
    in_maps = prep_inputs(x_q, scale_x, w_gate_up, s_gate_up, w_down, s_down, cfg)
    res = bass_utils.run_bass_kernel_spmd(nc, in_maps,
                                          core_ids=list(range(NCORES)))
    return assemble(res.results, cfg)


# revision 29
# speedup vs baseline: 1.1153x; 1.1153x over previous
"""Trainium2 Bass kernel for quantized Llama MLP (int8 gate_up -> silu*up ->
dynamic per-token requant -> int8 down_proj), tensor-parallel over 8 cores.

Sharding: column-parallel gate_up (2I split, gate/up halves aligned per shard),
row-parallel down (I split), AllReduce(max) for the dynamic per-token scale,
chunked ReduceScatter(add) on bf16 partial outputs.

The token dim is split in two halves, software-pipelined so the per-token
scale AllReduce + requantization of half h overlaps the matmuls of the next
phase; the PE stream is mm1(h0), mm1(h1), mm2(h0), mm2(h1) with no sync gaps.
"""
import sys, types
import numpy as np

if '/opt/trn_rl_repo' not in sys.path:
    sys.path.insert(0, '/opt/trn_rl_repo')

# antenv.axon_hooks is absent in this image; concourse imports it for NTFF
# profiling under axon. Register the ctypes-based hook before concourse loads.
def _ensure_ntff_hook():
    try:
        import antenv
        if "antenv.axon_hooks" not in sys.modules:
            hooks = types.ModuleType("antenv.axon_hooks")
            _h = [None]
            hooks.set_axon_ntff_profile_hook = lambda h: _h.__setitem__(0, h)
            hooks.get_axon_ntff_profile_hook = lambda: _h[0]
            sys.modules["antenv.axon_hooks"] = hooks
            antenv.axon_hooks = hooks
        import antenv.axon_hooks as hooks
        if hooks.get_axon_ntff_profile_hook() is None:
            try:
                from trn_agent_boot.trn_boot import _ntff_profile_via_ctypes
                hooks.set_axon_ntff_profile_hook(
                    _ntff_profile_via_ctypes('/opt/axon/libaxon_pjrt.so'))
            except Exception:
                pass
    except Exception:
        pass

_ensure_ntff_hook()

from concourse import bacc, tile, mybir
from concourse import bass_utils
from concourse import bass_isa

bass_utils.upload_artifacts = lambda tmpdir: tmpdir  # no bucket in container

F16 = mybir.dt.float16
F32 = mybir.dt.float32
BF16 = mybir.dt.bfloat16
MAGIC = 12582912.0  # 1.5 * 2**23: fp32 round-to-nearest-even at integers
# `a` is stored fp16 scaled by C_SCALE (folded into the up-proj scales) so
# silu(gate)*up products stay within fp16 range; round(a*127/amax) is
# invariant to the common factor, and the output dequant absorbs 1/C_SCALE.
C_SCALE = 1.0 / 64.0

NCORES = 8
MU = mybir.AluOpType.mult


class Cfg:
    def __init__(self, T, H, I, tb=512):
        self.T, self.H, self.I = T, H, I
        self.TB = tb                      # token block width for mm1 rhs
        self.NT = T // tb                 # mm1 t-blocks (even; split by halves)
        self.KH = H // 128                # mm1 contraction tiles
        nblk_tot = -(-I // 128)           # i 128-blocks, padded up
        nblk_tot = -(-nblk_tot // NCORES) * NCORES
        self.IPAD = nblk_tot * 128
        self.NPAIR = nblk_tot // NCORES   # i-blocks per core
        self.MT = T // 128                # mm2 token M-tiles (total)
        self.HB = min(512, H)             # mm2 h block
        self.HN = H // self.HB            # mm2 h N-blocks
        self.MPC = 2                      # m-tiles per ReduceScatter chunk
        self.MTH = self.MT // 2           # m-tiles per half
        self.NCHH = self.MTH // self.MPC  # RS chunks per half
        self.TCH = self.MPC * 128         # tokens per chunk
        self.TSL = self.TCH // NCORES     # tokens per rank slice
        self.T2 = T // 2
        assert self.NT % 2 == 0 and self.MTH % self.MPC == 0
        assert T % tb == 0 and H % 128 == 0


FULL = Cfg(2048, 4096, 11008)


def build(cfg=FULL, debug=False):
    T, H, I = cfg.T, cfg.H, cfg.I
    TB, NT, KH, NPAIR = cfg.TB, cfg.NT, cfg.KH, cfg.NPAIR
    MT, HB, HN = cfg.MT, cfg.HB, cfg.HN
    MPC, MTH, NCHH, TCH, TSL, T2 = (cfg.MPC, cfg.MTH, cfg.NCHH, cfg.TCH,
                                    cfg.TSL, cfg.T2)
    RG = [list(range(NCORES))]
    KHH = KH // 2
    NTH = NT // 2

    nc = bacc.Bacc("TRN2", target_bir_lowering=False, debug=debug,
                   num_devices=NCORES)
    xt_d = nc.dram_tensor("xt", [NT, 128, KH * TB], F16, kind="ExternalInput")
    wgu_d = nc.dram_tensor("wgu", [NPAIR, 2, 128, KH * 128], F16,
                           kind="ExternalInput")
    sgu_d = nc.dram_tensor("sgu", [128, NPAIR * 2], F32, kind="ExternalInput")
    wd_d = nc.dram_tensor("wd", [NPAIR, 128, H], F16, kind="ExternalInput")
    sx_d = nc.dram_tensor("sx", [1, T], F32, kind="ExternalInput")
    sdn_d = nc.dram_tensor("sdn", [1, H], F32, kind="ExternalInput")
    out_d = nc.dram_tensor("out", [T // NCORES, H], BF16, kind="ExternalOutput")

    with tile.TileContext(nc) as tc:
        with tc.tile_pool(name="const", bufs=1) as cpool, \
             tc.tile_pool(name="dram", bufs=1, space="DRAM") as dpool, \
             tc.tile_pool(name="psum", bufs=8, space="PSUM") as pspool, \
             tc.tile_pool(name="aq", bufs=1) as aqpool:

            sgu_sb = cpool.tile([128, NPAIR * 2], F32, tag="sgu")
            nc.sync.dma_start(out=sgu_sb[:, :], in_=sgu_d[:, :])
            sa_sb = cpool.tile([128, MT], F32, tag="sasb")
            inv_b = cpool.tile([128, T], F32, tag="invb")
            srow = [None, None]

            aq = aqpool.tile([128, NPAIR, T], F16, tag="aq")  # a then a_q

            s_loc_d = dpool.tile([1, T], F32, tag="sloc")
            s_glob_d = dpool.tile([1, T], F32, tag="sglob")
            chunk_d = [[dpool.tile([TCH, H], BF16, name=f"chunk{h}_{c}")
                        for c in range(NCHH)] for h in range(2)]
            rsout_d = [[dpool.tile([TSL, H], BF16, name=f"rsout{h}_{c}")
                        for c in range(NCHH)] for h in range(2)]

            def hsl(h):
                return slice(h * T2, (h + 1) * T2)

            NTH = NT // 2

            def quant_pair(h, pr):
                """a -> round(a * 127/amax) in place, for half h, pair pr."""
                for n in range(h * NTH, (h + 1) * NTH):
                    nsl = slice(n * TB, (n + 1) * TB)
                    tq = cpool.tile([128, TB], F32, tag="tq", bufs=2)
                    nc.vector.tensor_tensor(out=tq[:, :], in0=aq[:, pr, nsl],
                                            in1=inv_b[:, nsl], op=MU)
                    nc.vector.tensor_scalar(out=aq[:, pr, nsl], in0=tq[:, :],
                                            scalar1=MAGIC, scalar2=MAGIC,
                                            op0=mybir.AluOpType.add,
                                            op1=mybir.AluOpType.subtract)

            # ================= phase 1: gate_up + silu*up ==================
            with tc.tile_pool(name="xblk", bufs=3) as xpool, \
                 tc.tile_pool(name="wtile", bufs=4) as wpool, \
                 tc.tile_pool(name="tmp", bufs=2) as tpool, \
                 tc.tile_pool(name="rmax", bufs=1) as rpool:

                sx_row = rpool.tile([1, T], F32, tag="sxrow")
                nc.sync.dma_start(out=sx_row[:, :], in_=sx_d[:, :])
                sx_b = rpool.tile([128, T], F32, tag="sxb")
                nc.gpsimd.partition_broadcast(sx_b[:, :], sx_row[:, :])

                runmax = rpool.tile([128, T], F32, tag="runmax")
                nc.vector.memset(runmax[:, :], 0.0)
                runmin = rpool.tile([128, T], F32, tag="runmin")
                nc.vector.memset(runmin[:, :], 0.0)
                pmax = rpool.tile([128, T], F32, tag="pmax")

                def mm1_pair(pr, n, x_h):
                    """matmul gate+up for pair pr on t-block n + fused epilogue."""
                    nsl = slice(n * TB, (n + 1) * TB)
                    ps = []
                    for gu in range(2):
                        w_t = wpool.tile([128, KH, 128], F16, tag="w", bufs=4)
                        nc.sync.dma_start(out=w_t[:, :, :], in_=wgu_d[pr, gu])
                        p = pspool.tile([128, TB], F32, tag="ps")
                        for k in range(KH):
                            nc.tensor.matmul(p[:, :], lhsT=w_t[:, k, :],
                                             rhs=x_h[k // KHH][:, k % KHH, :],
                                             start=(k == 0), stop=(k == KH - 1))
                        ps.append(p)
                    g_ps, u_ps = ps
                    tg = tpool.tile([128, TB], F32, tag="tg")
                    nc.vector.tensor_tensor(out=tg[:, :], in0=g_ps[:, :],
                                            in1=sx_b[:, nsl], op=MU)
                    sig = tpool.tile([128, TB], F32, tag="sig")
                    nc.scalar.activation(sig[:, :], tg[:, :],
                                         mybir.ActivationFunctionType.Sigmoid,
                                         scale=sgu_sb[:, 2 * pr:2 * pr + 1])
                    tu = tpool.tile([128, TB], F32, tag="tu")
                    nc.vector.tensor_tensor(out=tu[:, :], in0=u_ps[:, :],
                                            in1=sx_b[:, nsl], op=MU)
                    t1 = tpool.tile([128, TB], F32, tag="t1")
                    nc.vector.scalar_tensor_tensor(
                        out=t1[:, :], in0=tu[:, :],
                        scalar=sgu_sb[:, 2 * pr + 1:2 * pr + 2],
                        in1=sig[:, :], op0=MU, op1=MU)
                    a_sl = aq[:, pr, nsl]
                    nc.vector.scalar_tensor_tensor(
                        out=a_sl, in0=tg[:, :],
                        scalar=sgu_sb[:, 2 * pr:2 * pr + 1],
                        in1=t1[:, :], op0=MU, op1=MU)
                    nc.vector.tensor_tensor(out=runmax[:, nsl],
                                            in0=runmax[:, nsl], in1=a_sl,
                                            op=mybir.AluOpType.max)
                    nc.vector.tensor_tensor(out=runmin[:, nsl],
                                            in0=runmin[:, nsl], in1=a_sl,
                                            op=mybir.AluOpType.min)

                def load_xhalves(n):
                    x_h = []
                    for q in range(2):
                        x_t = xpool.tile([128, KHH, TB], F16, tag="xt")
                        nc.sync.dma_start(
                            out=x_t[:, :, :],
                            in_=xt_d[n][:, q * KHH * TB:(q + 1) * KHH * TB])
                        x_h.append(x_t)
                    return x_h

                def sync_start(h):
                    """local partition amax -> AllReduce(max) for half h."""
                    s = hsl(h)
                    nc.gpsimd.partition_all_reduce(pmax[:, s], runmax[:, s],
                                                   128, bass_isa.ReduceOp.absmax)
                    nc.gpsimd.partition_all_reduce(runmax[:, s], runmin[:, s],
                                                   128, bass_isa.ReduceOp.absmax)
                    nc.vector.tensor_tensor(out=pmax[0:1, s], in0=pmax[0:1, s],
                                            in1=runmax[0:1, s],
                                            op=mybir.AluOpType.max)
                    nc.sync.dma_start(out=s_loc_d[0:1, s], in_=pmax[0:1, s])
                    nc.gpsimd.collective_compute(
                        "AllReduce", mybir.AluOpType.max, replica_groups=RG,
                        ins=[s_loc_d[0:1, s].opt()],
                        outs=[s_glob_d[0:1, s].opt()])
                    sr = cpool.tile([1, T2], F32, tag=f"srow{h}",
                                    name=f"srow{h}")
                    nc.sync.dma_start(out=sr[:, :], in_=s_glob_d[0:1, s])
                    srow[h] = sr
                    for m in range(h * MTH, (h + 1) * MTH):
                        nc.sync.dma_start(out=sa_sb[:, m:m + 1],
                                          in_=s_glob_d[0, 128 * m:128 * (m + 1)])

                def sync_finish(h):
                    """reciprocal + broadcast of 127/amax for half h."""
                    sr = srow[h]
                    nc.vector.reciprocal(sr[:, :], sr[:, :])
                    nc.vector.tensor_scalar(out=sr[:, :], in0=sr[:, :],
                                            scalar1=127.0, scalar2=None,
                                            op0=MU)
                    nc.gpsimd.partition_broadcast(inv_b[:, hsl(h)], sr[:, :])

                pass

                # ---- half 0 mm1 ----
                for n in range(NTH):
                    x_h = load_xhalves(n)
                    for pr in range(NPAIR):
                        mm1_pair(pr, n, x_h)
                sync_start(0)
                # ---- half 1 mm1, with half-0 scale+quant interleaved ----
                for n in range(NTH, NT):
                    x_h = load_xhalves(n)
                    for pr in range(NPAIR):
                        mm1_pair(pr, n, x_h)
                        if n == NTH:
                            if pr == 1:
                                sync_finish(0)
                            elif pr >= 2:
                                quant_pair(0, pr - 2)
                for pr in range(max(0, NPAIR - 2), NPAIR):
                    quant_pair(0, pr)
                sync_start(1)

            # ============== phase 2: down proj + ReduceScatter ==============
            with tc.tile_pool(name="wd", bufs=1) as wdpool, \
                 tc.tile_pool(name="ostage", bufs=1) as opool:
                sdn_row = wdpool.tile([1, H], F32, tag="sdnrow")
                nc.sync.dma_start(out=sdn_row[:, :], in_=sdn_d[:, :])
                sdn_b = wdpool.tile([128, H], F32, tag="sdnb")
                nc.gpsimd.partition_broadcast(sdn_b[:, :], sdn_row[:, :])
                wd_sb = wdpool.tile([128, NPAIR, H], F16, tag="wd")
                for blk in range(NPAIR):
                    nc.sync.dma_start(out=wd_sb[:, blk, :], in_=wd_d[blk])

                def mm2_mtile(h, c, mi):
                    m = h * MTH + c * MPC + mi
                    for n in range(HN):
                        ps = pspool.tile([128, HB], F32, tag="ps")
                        for blk in range(NPAIR):
                            nc.tensor.matmul(
                                ps[:, :],
                                lhsT=aq[:, blk, 128 * m:128 * (m + 1)],
                                rhs=wd_sb[:, blk, HB * n:HB * (n + 1)],
                                start=(blk == 0), stop=(blk == NPAIR - 1))
                        o = opool.tile([128, HB], BF16, tag="ost", bufs=8)
                        nc.vector.scalar_tensor_tensor(
                            out=o[:, :], in0=ps[:, :], scalar=sa_sb[:, m:m + 1],
                            in1=sdn_b[:, HB * n:HB * (n + 1)], op0=MU, op1=MU)
                        nc.sync.dma_start(
                            out=chunk_d[h][c][128 * mi:128 * (mi + 1),
                                              HB * n:HB * (n + 1)],
                            in_=o[:, :])

                def chunk_tail(h, c):
                    nc.gpsimd.collective_compute(
                        "ReduceScatter", mybir.AluOpType.add,
                        replica_groups=RG,
                        ins=[chunk_d[h][c][:, :].opt()],
                        outs=[rsout_d[h][c][:, :].opt()])
                    g = h * NCHH + c
                    nc.gpsimd.dma_start(out=out_d[g * TSL:(g + 1) * TSL, :],
                                        in_=rsout_d[h][c][:, :])

                # mm2 half 0, with half-1 scale+quant interleaved
                for c in range(NCHH):
                    for mi in range(MPC):
                        mm2_mtile(0, c, mi)
                        step = c * MPC + mi
                        if step == 1:
                            sync_finish(1)
                        elif step >= 2:
                            for pr in (2 * (step - 2), 2 * (step - 2) + 1):
                                if pr < NPAIR:
                                    quant_pair(1, pr)
                    chunk_tail(0, c)
                # finish any half-1 quant not covered by the interleave
                if MTH < 2:
                    sync_finish(1)
                for pr in range(min(NPAIR, max(0, 2 * (MTH - 2))), NPAIR):
                    quant_pair(1, pr)
                # mm2 half 1
                for c in range(NCHH):
                    for mi in range(MPC):
                        mm2_mtile(1, c, mi)
                    chunk_tail(1, c)

    nc.compile()
    return nc


def prep_inputs(x_q, scale_x, w_gate_up, s_gate_up, w_down, s_down, cfg=FULL):
    """Host-side shard + relayout + exact int8->fp16 cast. Returns in_maps."""
    T, H, I = cfg.T, cfg.H, cfg.I
    TB, NT, KH, NPAIR = cfg.TB, cfg.NT, cfg.KH, cfg.NPAIR
    IPAD = cfg.IPAD

    x_q = np.asarray(x_q); scale_x = np.asarray(scale_x, np.float32)
    w_gate_up = np.asarray(w_gate_up); s_gate_up = np.asarray(s_gate_up, np.float32)
    w_down = np.asarray(w_down); s_down = np.asarray(s_down, np.float32)

    # xt: [NT, 128, KH, TB] <- xT[h, t] = x_q[t, h]
    xt = np.ascontiguousarray(
        x_q.T.astype(np.float16).reshape(KH, 128, NT, TB).transpose(2, 1, 0, 3)
    ).reshape(NT, 128, KH * TB)

    def pad_rows(w, rows):
        return np.concatenate(
            [w, np.zeros((rows - w.shape[0],) + w.shape[1:], w.dtype)], 0) \
            if w.shape[0] < rows else w

    gate = pad_rows(w_gate_up[:I], IPAD)         # [IPAD, H] int8
    up = pad_rows(w_gate_up[I:], IPAD)
    s_g = pad_rows(s_gate_up[:I], IPAD)
    s_u = pad_rows(s_gate_up[I:], IPAD)
    wdp = np.concatenate(
        [w_down, np.zeros((H, IPAD - I), w_down.dtype)], 1)  # [H, IPAD]

    gate_b = gate.reshape(IPAD // 128, 128, H)
    up_b = up.reshape(IPAD // 128, 128, H)
    wd_b = np.ascontiguousarray(wdp.T).reshape(IPAD // 128, 128, H)

    sx = scale_x.reshape(1, T)
    sdn = (s_down / (127.0 * C_SCALE)).astype(np.float32).reshape(1, H)

    in_maps = []
    for k in range(NCORES):
        bsl = slice(k * NPAIR, (k + 1) * NPAIR)
        # wgu: [NPAIR, 2, 128(h_in), KH*128(o)]; lhsT tile [h_in, o]
        wgu_k = np.empty((NPAIR, 2, 128, KH, 128), np.float16)
        for j, blkset in enumerate((gate_b[bsl], up_b[bsl])):
            w = blkset.astype(np.float16).transpose(0, 2, 1)  # [NPAIR, H, 128]
            wgu_k[:, j] = w.reshape(NPAIR, KH, 128, 128).transpose(0, 2, 1, 3)
        sgu_k = np.empty((128, NPAIR * 2), np.float32)
        sgu_k[:, 0::2] = s_g[bsl.start * 128:bsl.stop * 128].reshape(NPAIR, 128).T
        sgu_k[:, 1::2] = (C_SCALE *
                          s_u[bsl.start * 128:bsl.stop * 128].reshape(NPAIR, 128).T)
        wd_k = wd_b[bsl].astype(np.float16)  # [NPAIR, 128(i_in), H]
        in_maps.append({
            "xt": xt.reshape(NT, 128, KH * TB),
            "wgu": np.ascontiguousarray(wgu_k).reshape(NPAIR, 2, 128, KH * 128),
            "sgu": sgu_k,
            "wd": np.ascontiguousarray(wd_k),
            "sx": sx, "sdn": sdn,
        })
    return in_maps


def assemble(results, cfg=FULL):
    T, H = cfg.T, cfg.H
    TSL, NCHH, TCH, T2 = cfg.TSL, cfg.NCHH, cfg.TCH, cfg.T2
    full = np.empty((T, H), np.float32)
    for k in range(NCORES):
        o = np.asarray(results[k]["out"]).astype(np.float32)
        for h in range(2):
            for c in range(NCHH):
                g = h * NCHH + c
                t0 = h * T2 + c * TCH + k * TSL
                full[t0:t0 + TSL] = o[g * TSL:(g + 1) * TSL]
    return full


_NC_CACHE = {}


def kernel(x_q, scale_x, w_gate_up, s_gate_up, w_down, s_down):
    cfg = FULL
    key = (cfg.T, cfg.H, cfg.I)
    if key not in _NC_CACHE:
        _NC_CACHE[key] = build(cfg)
    nc = _NC_CACHE[key]
    in_maps = prep_inputs(x_q, scale_x, w_gate_up, s_gate_up, w_down, s_down, cfg)
    res = bass_utils.run_bass_kernel_spmd(nc, in_maps,
                                          core_ids=list(range(NCORES)))
    return assemble(res.results, cfg)


# revision 31
# speedup vs baseline: 1.1196x; 1.0038x over previous
"""Trainium2 Bass kernel for quantized Llama MLP (int8 gate_up -> silu*up ->
dynamic per-token requant -> int8 down_proj), tensor-parallel over 8 cores.

Sharding: column-parallel gate_up (2I split, gate/up halves aligned per shard),
row-parallel down (I split), AllReduce(max) for the dynamic per-token scale,
chunked ReduceScatter(add) on bf16 partial outputs.

The token dim is split in two halves, software-pipelined so the per-token
scale AllReduce + requantization of half h overlaps the matmuls of the next
phase; the PE stream is mm1(h0), mm1(h1), mm2(h0), mm2(h1) with no sync gaps.
"""
import sys, types
import numpy as np

if '/opt/trn_rl_repo' not in sys.path:
    sys.path.insert(0, '/opt/trn_rl_repo')

# antenv.axon_hooks is absent in this image; concourse imports it for NTFF
# profiling under axon. Register the ctypes-based hook before concourse loads.
def _ensure_ntff_hook():
    try:
        import antenv
        if "antenv.axon_hooks" not in sys.modules:
            hooks = types.ModuleType("antenv.axon_hooks")
            _h = [None]
            hooks.set_axon_ntff_profile_hook = lambda h: _h.__setitem__(0, h)
            hooks.get_axon_ntff_profile_hook = lambda: _h[0]
            sys.modules["antenv.axon_hooks"] = hooks
            antenv.axon_hooks = hooks
        import antenv.axon_hooks as hooks
        if hooks.get_axon_ntff_profile_hook() is None:
            try:
                from trn_agent_boot.trn_boot import _ntff_profile_via_ctypes
                hooks.set_axon_ntff_profile_hook(
                    _ntff_profile_via_ctypes('/opt/axon/libaxon_pjrt.so'))
            except Exception:
                pass
    except Exception:
        pass

_ensure_ntff_hook()

from concourse import bacc, tile, mybir
from concourse import bass_utils
from concourse import bass_isa

bass_utils.upload_artifacts = lambda tmpdir: tmpdir  # no bucket in container

F16 = mybir.dt.float16
F32 = mybir.dt.float32
BF16 = mybir.dt.bfloat16
MAGIC = 12582912.0  # 1.5 * 2**23: fp32 round-to-nearest-even at integers
# `a` is stored fp16 scaled by C_SCALE (folded into the up-proj scales) so
# silu(gate)*up products stay within fp16 range; round(a*127/amax) is
# invariant to the common factor, and the output dequant absorbs 1/C_SCALE.
C_SCALE = 1.0 / 64.0

NCORES = 8
MU = mybir.AluOpType.mult


class Cfg:
    def __init__(self, T, H, I, tb=512):
        self.T, self.H, self.I = T, H, I
        self.TB = tb                      # token block width for mm1 rhs
        self.NT = T // tb                 # mm1 t-blocks (even; split by halves)
        self.KH = H // 128                # mm1 contraction tiles
        nblk_tot = -(-I // 128)           # i 128-blocks, padded up
        nblk_tot = -(-nblk_tot // NCORES) * NCORES
        self.IPAD = nblk_tot * 128
        self.NPAIR = nblk_tot // NCORES   # i-blocks per core
        self.MT = T // 128                # mm2 token M-tiles (total)
        self.HB = min(512, H)             # mm2 h block
        self.HN = H // self.HB            # mm2 h N-blocks
        self.MPC = 2                      # m-tiles per ReduceScatter chunk
        self.MTH = self.MT // 2           # m-tiles per half
        self.NCHH = self.MTH // self.MPC  # RS chunks per half
        self.TCH = self.MPC * 128         # tokens per chunk
        self.TSL = self.TCH // NCORES     # tokens per rank slice
        self.T2 = T // 2
        assert self.NT % 2 == 0 and self.MTH % self.MPC == 0
        assert T % tb == 0 and H % 128 == 0


FULL = Cfg(2048, 4096, 11008)


def build(cfg=FULL, debug=False):
    T, H, I = cfg.T, cfg.H, cfg.I
    TB, NT, KH, NPAIR = cfg.TB, cfg.NT, cfg.KH, cfg.NPAIR
    MT, HB, HN = cfg.MT, cfg.HB, cfg.HN
    MPC, MTH, NCHH, TCH, TSL, T2 = (cfg.MPC, cfg.MTH, cfg.NCHH, cfg.TCH,
                                    cfg.TSL, cfg.T2)
    RG = [list(range(NCORES))]
    KHH = KH // 2
    NTH = NT // 2

    nc = bacc.Bacc("TRN2", target_bir_lowering=False, debug=debug,
                   num_devices=NCORES)
    xt_d = nc.dram_tensor("xt", [NT, 128, KH * TB], F16, kind="ExternalInput")
    wgu_d = nc.dram_tensor("wgu", [NPAIR, 2, 128, KH * 128], F16,
                           kind="ExternalInput")
    sgu_d = nc.dram_tensor("sgu", [128, NPAIR * 2], F32, kind="ExternalInput")
    wd_d = nc.dram_tensor("wd", [NPAIR, 128, H], F16, kind="ExternalInput")
    sx_d = nc.dram_tensor("sx", [1, T], F32, kind="ExternalInput")
    sdn_d = nc.dram_tensor("sdn", [1, H], F32, kind="ExternalInput")
    out_d = nc.dram_tensor("out", [T // NCORES, H], BF16, kind="ExternalOutput")

    with tile.TileContext(nc) as tc:
        with tc.tile_pool(name="const", bufs=1) as cpool, \
             tc.tile_pool(name="dram", bufs=1, space="DRAM") as dpool, \
             tc.tile_pool(name="psum", bufs=8, space="PSUM") as pspool, \
             tc.tile_pool(name="aq", bufs=1) as aqpool:

            sgu_sb = cpool.tile([128, NPAIR * 2], F32, tag="sgu")
            nc.sync.dma_start(out=sgu_sb[:, :], in_=sgu_d[:, :])
            sa_sb = cpool.tile([128, MT], F32, tag="sasb")
            inv_b = cpool.tile([128, T], F32, tag="invb")
            srow = [None, None]

            aq = aqpool.tile([128, NPAIR, T], F16, tag="aq")  # a then a_q

            s_loc_d = dpool.tile([1, T], F32, tag="sloc")
            s_glob_d = dpool.tile([1, T], F32, tag="sglob")
            chunk_d = [[dpool.tile([TCH, H], BF16, name=f"chunk{h}_{c}")
                        for c in range(NCHH)] for h in range(2)]
            rsout_d = [[dpool.tile([TSL, H], BF16, name=f"rsout{h}_{c}")
                        for c in range(NCHH)] for h in range(2)]

            def hsl(h):
                return slice(h * T2, (h + 1) * T2)

            NTH = NT // 2

            def quant_pair(h, pr):
                """a -> round(a * 127/amax) in place, for half h, pair pr."""
                for n in range(h * NTH, (h + 1) * NTH):
                    nsl = slice(n * TB, (n + 1) * TB)
                    tq = cpool.tile([128, TB], F32, tag="tq", bufs=2)
                    nc.vector.tensor_tensor(out=tq[:, :], in0=aq[:, pr, nsl],
                                            in1=inv_b[:, nsl], op=MU)
                    nc.vector.tensor_scalar(out=aq[:, pr, nsl], in0=tq[:, :],
                                            scalar1=MAGIC, scalar2=MAGIC,
                                            op0=mybir.AluOpType.add,
                                            op1=mybir.AluOpType.subtract)

            # ================= phase 1: gate_up + silu*up ==================
            with tc.tile_pool(name="xblk", bufs=3) as xpool, \
                 tc.tile_pool(name="wtile", bufs=4) as wpool, \
                 tc.tile_pool(name="tmp", bufs=2) as tpool, \
                 tc.tile_pool(name="rmax", bufs=1) as rpool:

                sx_row = rpool.tile([1, T], F32, tag="sxrow")
                nc.sync.dma_start(out=sx_row[:, :], in_=sx_d[:, :])
                sx_b = rpool.tile([128, T], F32, tag="sxb")
                nc.gpsimd.partition_broadcast(sx_b[:, :], sx_row[:, :])

                runmax = rpool.tile([128, T], F32, tag="runmax")
                nc.vector.memset(runmax[:, :], 0.0)
                runmin = rpool.tile([128, T], F32, tag="runmin")
                nc.vector.memset(runmin[:, :], 0.0)
                pmax = rpool.tile([128, T], F32, tag="pmax")

                def mm1_pair(pr, n, x_h):
                    """matmul gate+up for pair pr on t-block n + fused epilogue."""
                    nsl = slice(n * TB, (n + 1) * TB)
                    ps = []
                    for gu in range(2):
                        w_t = wpool.tile([128, KH, 128], F16, tag="w", bufs=4)
                        nc.sync.dma_start(out=w_t[:, :, :], in_=wgu_d[pr, gu])
                        p = pspool.tile([128, TB], F32, tag="ps")
                        for k in range(KH):
                            nc.tensor.matmul(p[:, :], lhsT=w_t[:, k, :],
                                             rhs=x_h[k // KHH][:, k % KHH, :],
                                             start=(k == 0), stop=(k == KH - 1))
                        ps.append(p)
                    g_ps, u_ps = ps
                    tg = tpool.tile([128, TB], F32, tag="tg")
                    nc.vector.tensor_tensor(out=tg[:, :], in0=g_ps[:, :],
                                            in1=sx_b[:, nsl], op=MU)
                    sig = tpool.tile([128, TB], F32, tag="sig")
                    nc.scalar.activation(sig[:, :], tg[:, :],
                                         mybir.ActivationFunctionType.Sigmoid,
                                         scale=sgu_sb[:, 2 * pr:2 * pr + 1])
                    tu = tpool.tile([128, TB], F32, tag="tu")
                    nc.vector.tensor_tensor(out=tu[:, :], in0=u_ps[:, :],
                                            in1=sx_b[:, nsl], op=MU)
                    t1 = tpool.tile([128, TB], F32, tag="t1")
                    nc.vector.scalar_tensor_tensor(
                        out=t1[:, :], in0=tu[:, :],
                        scalar=sgu_sb[:, 2 * pr + 1:2 * pr + 2],
                        in1=sig[:, :], op0=MU, op1=MU)
                    a_sl = aq[:, pr, nsl]
                    nc.vector.scalar_tensor_tensor(
                        out=a_sl, in0=tg[:, :],
                        scalar=sgu_sb[:, 2 * pr:2 * pr + 1],
                        in1=t1[:, :], op0=MU, op1=MU)
                    nc.vector.tensor_tensor(out=runmax[:, nsl],
                                            in0=runmax[:, nsl], in1=a_sl,
                                            op=mybir.AluOpType.max)
                    nc.vector.tensor_tensor(out=runmin[:, nsl],
                                            in0=runmin[:, nsl], in1=a_sl,
                                            op=mybir.AluOpType.min)

                def load_xhalves(n):
                    x_h = []
                    for q in range(2):
                        x_t = xpool.tile([128, KHH, TB], F16, tag="xt")
                        nc.sync.dma_start(
                            out=x_t[:, :, :],
                            in_=xt_d[n][:, q * KHH * TB:(q + 1) * KHH * TB])
                        x_h.append(x_t)
                    return x_h

                def sync_start(h):
                    """local partition amax -> AllReduce(max) for half h."""
                    s = hsl(h)
                    nc.gpsimd.partition_all_reduce(pmax[:, s], runmax[:, s],
                                                   128, bass_isa.ReduceOp.absmax)
                    nc.gpsimd.partition_all_reduce(runmax[:, s], runmin[:, s],
                                                   128, bass_isa.ReduceOp.absmax)
                    nc.vector.tensor_tensor(out=pmax[0:1, s], in0=pmax[0:1, s],
                                            in1=runmax[0:1, s],
                                            op=mybir.AluOpType.max)
                    nc.sync.dma_start(out=s_loc_d[0:1, s], in_=pmax[0:1, s])
                    nc.gpsimd.collective_compute(
                        "AllReduce", mybir.AluOpType.max, replica_groups=RG,
                        ins=[s_loc_d[0:1, s].opt()],
                        outs=[s_glob_d[0:1, s].opt()])
                    sr = cpool.tile([1, T2], F32, tag=f"srow{h}",
                                    name=f"srow{h}")
                    nc.sync.dma_start(out=sr[:, :], in_=s_glob_d[0:1, s])
                    srow[h] = sr
                    for m in range(h * MTH, (h + 1) * MTH):
                        nc.sync.dma_start(out=sa_sb[:, m:m + 1],
                                          in_=s_glob_d[0, 128 * m:128 * (m + 1)])

                def sync_finish(h):
                    """reciprocal + broadcast of 127/amax for half h."""
                    sr = srow[h]
                    nc.vector.reciprocal(sr[:, :], sr[:, :])
                    nc.vector.tensor_scalar(out=sr[:, :], in0=sr[:, :],
                                            scalar1=127.0, scalar2=None,
                                            op0=MU)
                    nc.gpsimd.partition_broadcast(inv_b[:, hsl(h)], sr[:, :])

                pass

                # ---- half 0 mm1 ----
                for n in range(NTH):
                    x_h = load_xhalves(n)
                    for pr in range(NPAIR):
                        mm1_pair(pr, n, x_h)
                sync_start(0)
                # ---- half 1 mm1, with half-0 scale+quant interleaved ----
                FIN0 = min(3, NPAIR - 1)
                for n in range(NTH, NT):
                    x_h = load_xhalves(n)
                    for pr in range(NPAIR):
                        mm1_pair(pr, n, x_h)
                        if n == NTH:
                            if pr == FIN0:
                                sync_finish(0)
                            elif pr > FIN0:
                                quant_pair(0, pr - FIN0 - 1)
                for pr in range(max(0, NPAIR - FIN0 - 1), NPAIR):
                    quant_pair(0, pr)
                sync_start(1)

            # ============== phase 2: down proj + ReduceScatter ==============
            with tc.tile_pool(name="wd", bufs=1) as wdpool, \
                 tc.tile_pool(name="ostage", bufs=1) as opool:
                sdn_row = wdpool.tile([1, H], F32, tag="sdnrow")
                nc.sync.dma_start(out=sdn_row[:, :], in_=sdn_d[:, :])
                sdn_b = wdpool.tile([128, H], F32, tag="sdnb")
                nc.gpsimd.partition_broadcast(sdn_b[:, :], sdn_row[:, :])
                wd_sb = wdpool.tile([128, NPAIR, H], F16, tag="wd")
                for blk in range(NPAIR):
                    nc.sync.dma_start(out=wd_sb[:, blk, :], in_=wd_d[blk])

                def mm2_mtile(h, c, mi):
                    m = h * MTH + c * MPC + mi
                    for n in range(HN):
                        ps = pspool.tile([128, HB], F32, tag="ps")
                        for blk in range(NPAIR):
                            nc.tensor.matmul(
                                ps[:, :],
                                lhsT=aq[:, blk, 128 * m:128 * (m + 1)],
                                rhs=wd_sb[:, blk, HB * n:HB * (n + 1)],
                                start=(blk == 0), stop=(blk == NPAIR - 1))
                        o = opool.tile([128, HB], BF16, tag="ost", bufs=8)
                        nc.vector.scalar_tensor_tensor(
                            out=o[:, :], in0=ps[:, :], scalar=sa_sb[:, m:m + 1],
                            in1=sdn_b[:, HB * n:HB * (n + 1)], op0=MU, op1=MU)
                        nc.sync.dma_start(
                            out=chunk_d[h][c][128 * mi:128 * (mi + 1),
                                              HB * n:HB * (n + 1)],
                            in_=o[:, :])

                def chunk_tail(h, c):
                    nc.gpsimd.collective_compute(
                        "ReduceScatter", mybir.AluOpType.add,
                        replica_groups=RG,
                        ins=[chunk_d[h][c][:, :].opt()],
                        outs=[rsout_d[h][c][:, :].opt()])
                    g = h * NCHH + c
                    nc.gpsimd.dma_start(out=out_d[g * TSL:(g + 1) * TSL, :],
                                        in_=rsout_d[h][c][:, :])

                # mm2 half 0, with half-1 scale+quant interleaved
                FIN1 = min(3, MTH - 1)
                for c in range(NCHH):
                    for mi in range(MPC):
                        mm2_mtile(0, c, mi)
                        step = c * MPC + mi
                        if step == FIN1:
                            sync_finish(1)
                        elif step > FIN1:
                            for pr in (2 * (step - FIN1 - 1),
                                       2 * (step - FIN1 - 1) + 1):
                                if pr < NPAIR:
                                    quant_pair(1, pr)
                    chunk_tail(0, c)
                # finish any half-1 quant not covered by the interleave
                if MTH - 1 < FIN1 or MTH < 1:
                    sync_finish(1)
                for pr in range(min(NPAIR, max(0, 2 * (MTH - 1 - FIN1))), NPAIR):
                    quant_pair(1, pr)
                # mm2 half 1
                for c in range(NCHH):
                    for mi in range(MPC):
                        mm2_mtile(1, c, mi)
                    chunk_tail(1, c)

    nc.compile()
    return nc


def prep_inputs(x_q, scale_x, w_gate_up, s_gate_up, w_down, s_down, cfg=FULL):
    """Host-side shard + relayout + exact int8->fp16 cast. Returns in_maps."""
    T, H, I = cfg.T, cfg.H, cfg.I
    TB, NT, KH, NPAIR = cfg.TB, cfg.NT, cfg.KH, cfg.NPAIR
    IPAD = cfg.IPAD

    x_q = np.asarray(x_q); scale_x = np.asarray(scale_x, np.float32)
    w_gate_up = np.asarray(w_gate_up); s_gate_up = np.asarray(s_gate_up, np.float32)
    w_down = np.asarray(w_down); s_down = np.asarray(s_down, np.float32)

    # xt: [NT, 128, KH, TB] <- xT[h, t] = x_q[t, h]
    xt = np.ascontiguousarray(
        x_q.T.astype(np.float16).reshape(KH, 128, NT, TB).transpose(2, 1, 0, 3)
    ).reshape(NT, 128, KH * TB)

    def pad_rows(w, rows):
        return np.concatenate(
            [w, np.zeros((rows - w.shape[0],) + w.shape[1:], w.dtype)], 0) \
            if w.shape[0] < rows else w

    gate = pad_rows(w_gate_up[:I], IPAD)         # [IPAD, H] int8
    up = pad_rows(w_gate_up[I:], IPAD)
    s_g = pad_rows(s_gate_up[:I], IPAD)
    s_u = pad_rows(s_gate_up[I:], IPAD)
    wdp = np.concatenate(
        [w_down, np.zeros((H, IPAD - I), w_down.dtype)], 1)  # [H, IPAD]

    gate_b = gate.reshape(IPAD // 128, 128, H)
    up_b = up.reshape(IPAD // 128, 128, H)
    wd_b = np.ascontiguousarray(wdp.T).reshape(IPAD // 128, 128, H)

    sx = scale_x.reshape(1, T)
    sdn = (s_down / (127.0 * C_SCALE)).astype(np.float32).reshape(1, H)

    in_maps = []
    for k in range(NCORES):
        bsl = slice(k * NPAIR, (k + 1) * NPAIR)
        # wgu: [NPAIR, 2, 128(h_in), KH*128(o)]; lhsT tile [h_in, o]
        wgu_k = np.empty((NPAIR, 2, 128, KH, 128), np.float16)
        for j, blkset in enumerate((gate_b[bsl], up_b[bsl])):
            w = blkset.astype(np.float16).transpose(0, 2, 1)  # [NPAIR, H, 128]
            wgu_k[:, j] = w.reshape(NPAIR, KH, 128, 128).transpose(0, 2, 1, 3)
        sgu_k = np.empty((128, NPAIR * 2), np.float32)
        sgu_k[:, 0::2] = s_g[bsl.start * 128:bsl.stop * 128].reshape(NPAIR, 128).T
        sgu_k[:, 1::2] = (C_SCALE *
                          s_u[bsl.start * 128:bsl.stop * 128].reshape(NPAIR, 128).T)
        wd_k = wd_b[bsl].astype(np.float16)  # [NPAIR, 128(i_in), H]
        in_maps.append({
            "xt": xt.reshape(NT, 128, KH * TB),
            "wgu": np.ascontiguousarray(wgu_k).reshape(NPAIR, 2, 128, KH * 128),
            "sgu": sgu_k,
            "wd": np.ascontiguousarray(wd_k),
            "sx": sx, "sdn": sdn,
        })
    return in_maps


def assemble(results, cfg=FULL):
    T, H = cfg.T, cfg.H
    TSL, NCHH, TCH, T2 = cfg.TSL, cfg.NCHH, cfg.TCH, cfg.T2
    full = np.empty((T, H), np.float32)
    for k in range(NCORES):
        o = np.asarray(results[k]["out"]).astype(np.float32)
        for h in range(2):
            for c in range(NCHH):
                g = h * NCHH + c
                t0 = h * T2 + c * TCH + k * TSL
                full[t0:t0 + TSL] = o[g * TSL:(g + 1) * TSL]
    return full


_NC_CACHE = {}


def kernel(x_q, scale_x, w_gate_up, s_gate_up, w_down, s_down):
    cfg = FULL
    key = (cfg.T, cfg.H, cfg.I)
    if key not in _NC_CACHE:
        _NC_CACHE[key] = build(cfg)
    nc = _NC_CACHE[key]
    in_maps = prep_inputs(x_q, scale_x, w_gate_up, s_gate_up, w_down, s_down, cfg)
    res = bass_utils.run_bass_kernel_spmd(nc, in_maps,
                                          core_ids=list(range(NCORES)))
    return assemble(res.results, cfg)
